# revision 37
# baseline (speedup 1.0000x reference)
"""Distributed GQA attention kernel for one TRN2 chip (8 NeuronCores).

Problem: B=1, Q=1024 new tokens, C=3072 cached, H=4096, 32 Q heads,
8 KV heads, head_dim=128.  Tensor-parallel over heads: core c owns
Q heads 4c..4c+3 and KV head c.  q/k/v projections column-sharded,
o_proj column-sharded with an AllGather of per-head attention outputs
(equivalent wire cost to row-shard + all-reduce, but transport only —
no CCE arithmetic).  Compute in bf16 (PE runs bf16 at 4x the fp32
rate); accumulation is fp32 in PSUM.

Layout strategy (all host-side prep is free):
  - hidden^T [H+1(bias row), T] so projections contract over partitions
  - past_key^T [hd, kp];  past_value pre-tiled [128, 24, 128]
  - scores computed transposed: s^T[kp, t] = K^T(lhsT) . Q^T(rhs)
  - softmax without max subtraction (scores ~ N(0, ~3.3), exp never
    overflows fp32) and with mask folded in as exp(mask) (host
    precomputed), multiplied post-exp on DVE
  - denominators via ones-matmul on the PE, broadcast back over
    partitions via a 1-partition matmul
"""

import sys

if "/opt/trn_rl_repo" not in sys.path:
    sys.path.insert(0, "/opt/trn_rl_repo")

import numpy as np
import ml_dtypes

BF16 = ml_dtypes.bfloat16

B, Q, C = 1, 1024, 3072
H, NH, KVH = 4096, 32, 8
HD = 128
S = C + Q                  # 4096 key positions
NCORES = 8
HPC = NH // NCORES         # 4 q heads per core
KT = H // 128 + 1          # 33 contraction tiles (incl. bias row tile)
NKP = S // 128             # 32 key-position tiles
CKP = C // 128             # 24 past kv tiles
NTT = Q // 128             # 8 new-token tiles
ISC = 1.0 / np.sqrt(HD)

_CACHE = {}


def _build():
    from concourse import bass, bacc, tile, mybir

    f32 = mybir.dt.float32
    bf16 = mybir.dt.bfloat16
    AF = mybir.ActivationFunctionType

    nc = bacc.Bacc("TRN2", target_bir_lowering=False, debug=False,
                   num_devices=NCORES)

    # ---- I/O ----
    hx_d = nc.dram_tensor("hx", [KT, 128, Q], bf16, kind="ExternalInput")
    wq_d = nc.dram_tensor("wq", [KT, 128, HPC * HD], bf16, kind="ExternalInput")
    wk_d = nc.dram_tensor("wk", [KT, 128, HD], bf16, kind="ExternalInput")
    wv_d = nc.dram_tensor("wv", [KT, 128, HD], bf16, kind="ExternalInput")
    wo_d = nc.dram_tensor("wo", [NH, 128, HPC * HD], bf16, kind="ExternalInput")
    bo_d = nc.dram_tensor("bo_col", [128, HPC], f32, kind="ExternalInput")
    pk_d = nc.dram_tensor("pkT", [128, C], bf16, kind="ExternalInput")
    pv_d = nc.dram_tensor("pv", [128, CKP, 128], bf16, kind="ExternalInput")
    cos_d = nc.dram_tensor("cosT", [128, Q], bf16, kind="ExternalInput")
    sin_d = nc.dram_tensor("ssT", [128, Q], bf16, kind="ExternalInput")
    em_d = nc.dram_tensor("expmT", [NKP, 128, Q], bf16, kind="ExternalInput")
    id_d = nc.dram_tensor("ident", [128, 128], bf16, kind="ExternalInput")

    attn_d = nc.dram_tensor("attn_t", [HPC, 128, Q], f32, kind="ExternalOutput")
    knew_d = nc.dram_tensor("k_new", [128, Q], f32, kind="ExternalOutput")
    vnew_d = nc.dram_tensor("v_new", [128, Q], f32, kind="ExternalOutput")

    with tile.TileContext(nc) as tc:
        with (
            tc.tile_pool(name="const", bufs=1) as cpool,
            tc.tile_pool(name="proj", bufs=1) as proj,
            tc.tile_pool(name="ps", bufs=1, space="PSUM") as ps,
            tc.tile_pool(name="dram", bufs=1, space="DRAM") as dram,
        ):
            # ---- constants (DMAs issued later, behind the hot-path tiles) --
            cos_sb = cpool.tile([128, Q], bf16)
            ss_sb = cpool.tile([128, Q], bf16)
            bo_sb = cpool.tile([128, HPC], f32)
            ident = cpool.tile([128, 128], bf16)
            ones_sb = cpool.tile([128, 1], bf16)
            onesf_sb = cpool.tile([1, 128], bf16)
            zbias = cpool.tile([128, 1], f32)
            nc.vector.memset(ones_sb[:], 1.0)
            nc.vector.memset(onesf_sb[:], 1.0)
            nc.vector.memset(zbias[:], 0.0)

            # ---- persistent per-core tensors ----
            qT_sb = proj.tile([128, HPC, Q], bf16)      # rope'd Q^T per head
            kT_sb = proj.tile([128, S], bf16)           # full K^T (past+new)
            v_sb = proj.tile([128, NKP, 128], bf16)     # full V, kp-tiled

            # ================= phase 1: projections + RoPE =================
            # hidden^T is fully SBUF-resident: streamed in once during the
            # Q pass, reused by the K/V pass with zero extra HBM traffic.
            with (
                tc.tile_pool(name="hxs", bufs=1) as hxp,
                tc.tile_pool(name="wqs", bufs=6) as wqp,
                tc.tile_pool(name="wkvs", bufs=6) as wkvp,
                tc.tile_pool(name="rope", bufs=2) as rp,
            ):
                hx_sb = hxp.tile([128, KT, Q], bf16)

                def rope(dst_bf, ps, also_f32=None):
                    """dst = ps*cos + rotate_half(ps)*ss  (ps is [128, Q] psum)"""
                    a = rp.tile([128, Q], bf16, name="ropa", tag="ropa")
                    b = rp.tile([128, Q], bf16, name="ropb", tag="ropb")
                    nc.vector.tensor_mul(a[:], ps[:], cos_sb[:])
                    nc.vector.tensor_mul(b[0:64, :], ps[64:128, :], ss_sb[0:64, :])
                    nc.vector.tensor_mul(b[64:128, :], ps[0:64, :], ss_sb[64:128, :])
                    nc.vector.tensor_add(dst_bf, a[:], b[:])
                    if also_f32 is not None:
                        nc.vector.tensor_add(also_f32, a[:], b[:])

                # ---- pass A: Q projection, all 4 heads per k-tile ----
                q_ps = [ps.tile([128, Q], f32, name=f"qps{m}", tag=f"S{m}")
                        for m in range(HPC)]
                for k in range(KT):
                    nc.sync.dma_start(hx_sb[:, k, :], hx_d.ap()[k, :, :])
                    wqk = wqp.tile([128, HPC * HD], bf16, name="wqk",
                                   tag="wqk")
                    nc.sync.dma_start(wqk[:], wq_d.ap()[k, :, :])
                    for m in range(HPC):
                        for th in range(2):
                            nc.tensor.matmul(
                                q_ps[m][:, th * 512:(th + 1) * 512],
                                wqk[:, m * 128:(m + 1) * 128],
                                hx_sb[:, k, th * 512:(th + 1) * 512],
                                start=(k == 0), stop=(k == KT - 1))
                # constants + kv cache loads, behind the hot startup path
                nc.sync.dma_start(cos_sb[:], cos_d.ap()[:])
                nc.sync.dma_start(ss_sb[:], sin_d.ap()[:])
                nc.sync.dma_start(ident[:], id_d.ap()[:])
                nc.sync.dma_start(bo_sb[:], bo_d.ap()[:])
                nc.sync.dma_start(kT_sb[:, 0:C], pk_d.ap()[:])
                nc.sync.dma_start(v_sb[:, 0:CKP, :], pv_d.ap()[:])
                for m in range(HPC):
                    rope(qT_sb[:, m, :], q_ps[m])

                # ---- pass B: K and V projections ----
                k_ps = ps.tile([128, Q], f32, name="kps", tag="S0")
                v_ps = ps.tile([128, Q], f32, name="vps", tag="S1")
                for k in range(KT):
                    wkv = wkvp.tile([128, 2 * HD], bf16, name="wkv",
                                    tag="wkv")
                    nc.sync.dma_start(wkv[:, 0:HD], wk_d.ap()[k, :, :])
                    nc.sync.dma_start(wkv[:, HD:2 * HD], wv_d.ap()[k, :, :])
                    for th in range(2):
                        sl = slice(th * 512, (th + 1) * 512)
                        nc.tensor.matmul(k_ps[:, sl], wkv[:, 0:HD],
                                         hx_sb[:, k, sl],
                                         start=(k == 0), stop=(k == KT - 1))
                        nc.tensor.matmul(v_ps[:, sl], wkv[:, HD:2 * HD],
                                         hx_sb[:, k, sl],
                                         start=(k == 0), stop=(k == KT - 1))
                # V path first: the PE's in-order queue reaches the
                # transposes (and then head 0's scores) without waiting on
                # the K-RoPE DVE chain; K^T's new region isn't read until
                # kp tile 24, well into phase 2.
                vT_bf = rp.tile([128, Q], bf16)
                nc.vector.tensor_copy(vT_bf[:], v_ps[:])
                vnew_f = rp.tile([128, Q], f32)
                nc.vector.tensor_copy(vnew_f[:], v_ps[:])
                nc.sync.dma_start(vnew_d.ap()[:], vnew_f[:])
                for j in range(NTT):
                    tp = ps.tile([128, 128], bf16, name="vtp",
                                 tag=f"S{2 + (j % 2)}")
                    nc.tensor.transpose(
                        tp[:], vT_bf[:, j * 128:(j + 1) * 128], ident[:])
                    nc.vector.tensor_copy(v_sb[:, CKP + j, :], tp[:])

                knew_f = rp.tile([128, Q], f32)
                rope(kT_sb[:, C:S], k_ps, also_f32=knew_f[:])
                nc.sync.dma_start(knew_d.ap()[:], knew_f[:])

            # exp(mask) reuses the SBUF freed by hx; streamed in at the
            # start of phase 2 (phase 2 is PE-bound with DMA headroom),
            # kp-ordered so head 0 stays ahead of the arrivals.
            emres = tc.alloc_tile_pool(name="emres", bufs=1)
            em_sb = emres.tile([128, NKP, Q], bf16)
            for kp in range(NKP):
                nc.sync.dma_start(em_sb[:, kp, :], em_d.ap()[kp, :, :])

            # first AllGather half lands in SBUF mid-phase-2 (right-side
            # stack so it outlives emres, which releases before phase 3)
            agA = tc.alloc_tile_pool(name="agA", bufs=1, side="right")
            agA_sb = agA.tile([128, NH // 2, Q], bf16)

            # ================= phase 2: attention =================
            # AllGather is split in two: heads {0,1} are gathered while
            # heads {2,3} still compute, the second gather overlaps the
            # start of o_proj.
            attg_in = [dram.tile([2, 128, Q], bf16, name=f"aggi{i}")
                       for i in range(2)]
            attg_all = [dram.tile([NH // 2, 128, Q], bf16,
                                  addr_space="Shared", name=f"agga{i}")
                        for i in range(2)]
            att_sb = proj.tile([128, HPC, Q], bf16)
            with (
                tc.tile_pool(name="p2sb", bufs=4) as p2sb,
                tc.tile_pool(name="p2misc", bufs=2) as p2m,
            ):
                for h in range(HPC):
                    av_ps = ps.tile([128, Q], f32, name="av", tag="S1")
                    den_ps = ps.tile([1, Q], f32, name="den", tag="S0")
                    pw_even = None
                    for kp in range(NKP):
                        sc = ps.tile([128, Q], f32, name="sc",
                                     tag=f"S{2 + (kp % 2)}")
                        for th in range(2):
                            nc.tensor.matmul(
                                sc[:, th * 512:(th + 1) * 512],
                                kT_sb[:, kp * 128:(kp + 1) * 128],
                                qT_sb[:, h, th * 512:(th + 1) * 512],
                                start=True, stop=True)
                        pexp = p2sb.tile([128, Q], bf16, name="pexp",
                                         tag="pexp")
                        nc.scalar.activation(pexp[:], sc[:], AF.Exp,
                                             bias=zbias[:], scale=ISC)
                        pw = p2sb.tile([128, Q], bf16, name="pw", tag="pw")
                        nc.vector.tensor_mul(pw[:], pexp[:], em_sb[:, kp, :])
                        first, last = kp == 0, kp == NKP - 1
                        for th in range(2):
                            sl = slice(th * 512, (th + 1) * 512)
                            nc.tensor.matmul(av_ps[:, sl],
                                             v_sb[:, kp, :], pw[:, sl],
                                             start=first, stop=last)
                        if kp % 2 == 0:
                            pw_even = pw
                        else:
                            # denominator: tree-sum pw quads on DVE, then one
                            # ones-matmul per quad (quarters the PE cost)
                            pws = p2sb.tile([128, Q], bf16, name="pws",
                                            tag="pws")
                            nc.vector.tensor_add(pws[:], pw_even[:], pw[:])
                            if kp % 4 == 1:
                                pws_prev = pws
                            else:
                                qws = p2sb.tile([128, Q], bf16, name="qws",
                                                tag="qws")
                                nc.vector.tensor_add(qws[:], pws_prev[:],
                                                     pws[:])
                                for th in range(2):
                                    sl = slice(th * 512, (th + 1) * 512)
                                    nc.tensor.matmul(den_ps[:, sl],
                                                     ones_sb[:], qws[:, sl],
                                                     start=(kp == 3),
                                                     stop=(kp == NKP - 1))
                    # normalize: att = av / den
                    recip = p2m.tile([1, Q], f32, name="recip", tag="recip")
                    nc.vector.reciprocal_approx_fast(recip[:], den_ps[:])
                    recipb = p2m.tile([1, Q], bf16, name="recipb",
                                      tag="recipb")
                    nc.vector.tensor_copy(recipb[:], recip[:])
                    rb_ps = ps.tile([128, Q], f32, name="rb", tag="S0")
                    for th in range(2):
                        nc.tensor.matmul(
                            rb_ps[:, th * 512:(th + 1) * 512],
                            onesf_sb[:],
                            recipb[:, th * 512:(th + 1) * 512],
                            start=True, stop=True)
                    rb_sb = p2m.tile([128, Q], bf16, name="rbsb", tag="rbsb")
                    nc.vector.tensor_copy(rb_sb[:], rb_ps[:])
                    nc.vector.tensor_mul(att_sb[:, h, :], av_ps[:], rb_sb[:])
                    # kick off the half-gather as soon as its heads are done
                    if h == 1 or h == HPC - 1:
                        half = 0 if h == 1 else 1
                        for hi in range(2):
                            nc.sync.dma_start(
                                attg_in[half][hi, :, :],
                                att_sb[:, 2 * half + hi, :])
                        nc.gpsimd.collective_compute(
                            "AllGather",
                            mybir.AluOpType.bypass,
                            replica_groups=[list(range(NCORES))],
                            ins=[attg_in[half].opt()],
                            outs=[attg_all[half].opt()],
                        )
                        if half == 0:
                            # pull gather-half A into SBUF while heads 2-3
                            # still compute
                            for j in range(NH // 2):
                                nc.sync.dma_start(agA_sb[:, j, :],
                                                  attg_all[0][j, :, :])
            emres.release()

            # ================= phase 3: o_proj =================
            # global block j of half i holds head 4*(j//2) + 2*i + j%2;
            # the host reorders Wo blocks to match, so we just iterate.
            with (
                tc.tile_pool(name="p3sb", bufs=1) as p3sb,
                tc.tile_pool(name="wos", bufs=6) as wop,
            ):
                agB_sb = p3sb.tile([128, NH // 2, Q], bf16)
                out_sb = p3sb.tile([128, HPC, Q], f32)
                for j in range(NH // 2):
                    nc.gpsimd.dma_start(agB_sb[:, j, :], attg_all[1][j, :, :])
                op = [ps.tile([128, Q], f32, name=f"op{m}", tag=f"S{m}")
                      for m in range(HPC)]
                for g in range(NH):
                    wog = wop.tile([128, HPC * HD], bf16, name="wog",
                                   tag="wog")
                    nc.sync.dma_start(wog[:], wo_d.ap()[g, :, :])
                    src = agA_sb[:, g, :] if g < NH // 2 \
                        else agB_sb[:, g - NH // 2, :]
                    for m in range(HPC):
                        for th in range(2):
                            sl = slice(th * 512, (th + 1) * 512)
                            nc.tensor.matmul(
                                op[m][:, sl],
                                wog[:, m * 128:(m + 1) * 128],
                                src[:, sl],
                                start=(g == 0), stop=(g == NH - 1))
                for m in range(HPC):
                    nc.scalar.activation(out_sb[:, m, :], op[m][:],
                                         AF.Identity,
                                         bias=bo_sb[:, m:m + 1], scale=1.0)
                    nc.sync.dma_start(attn_d.ap()[m, :, :], out_sb[:, m, :])
            agA.release()

    nc.compile()
    return nc


def _get_nc():
    if "nc" not in _CACHE:
        _CACHE["nc"] = _build()
    return _CACHE["nc"]


def kernel(hidden_states, mask, pos_emb, past_key, past_value,
           Wq, bq, Wk, bk, Wv, bv, Wo, bo):
    from concourse.bass_utils import run_bass_kernel_spmd

    hidden_states = np.asarray(hidden_states, np.float32)
    mask = np.asarray(mask, np.float32)
    pos_emb = np.asarray(pos_emb, np.float32)
    past_key = np.asarray(past_key, np.float32)
    past_value = np.asarray(past_value, np.float32)
    Wq = np.asarray(Wq, np.float32)
    bq = np.asarray(bq, np.float32)
    Wk = np.asarray(Wk, np.float32)
    bk = np.asarray(bk, np.float32)
    Wv = np.asarray(Wv, np.float32)
    bv = np.asarray(bv, np.float32)
    Wo = np.asarray(Wo, np.float32)
    bo = np.asarray(bo, np.float32)

    # ---- shared host prep ----
    hx = np.zeros((KT * 128, Q), np.float32)
    hx[:H] = hidden_states[0].T
    hx[H] = 1.0                                   # bias row
    hx = hx.reshape(KT, 128, Q).astype(BF16)

    cosT = pos_emb[0, 0].T.astype(BF16)           # [128, Q]
    sinT = pos_emb[0, 1].T
    ssT = np.concatenate([-sinT[:64], sinT[64:]]).astype(BF16)

    expmT = np.exp(mask[0, 0]).T.reshape(NKP, 128, Q).astype(BF16)

    # o_proj iterates gathered blocks: g -> head 4*((g%16)//2) + 2*(g//16) + g%2
    wo_order = [4 * ((g % 16) // 2) + 2 * (g // 16) + g % 2 for g in range(NH)]

    def wpad(w, b):
        x = np.zeros((KT * 128, w.shape[1]), np.float32)
        x[:H] = w
        x[H] = b
        return x.reshape(KT, 128, w.shape[1]).astype(BF16)

    in_maps = []
    for c in range(NCORES):
        qs = slice(c * HPC * HD, (c + 1) * HPC * HD)
        ks = slice(c * HD, (c + 1) * HD)
        in_maps.append({
            "hx": hx,
            "wq": wpad(Wq[:, qs], bq[qs]),
            "wk": wpad(Wk[:, ks], bk[ks]),
            "wv": wpad(Wv[:, ks], bv[ks]),
            "wo": Wo[:, qs].reshape(NH, 128, HPC * HD)[wo_order].astype(BF16),
            "bo_col": np.ascontiguousarray(
                bo[qs].reshape(HPC, 128).T.astype(np.float32)),
            "pkT": np.ascontiguousarray(past_key[0, c].T).astype(BF16),
            "pv": np.ascontiguousarray(
                past_value[0, c].reshape(CKP, 128, HD).transpose(1, 0, 2)
            ).astype(BF16),
            "cosT": cosT,
            "ssT": ssT,
            "expmT": expmT,
            "ident": np.eye(128, dtype=BF16),
        })

    nc = _get_nc()
    res = run_bass_kernel_spmd(nc, in_maps, core_ids=list(range(NCORES)))

    # ---- assemble full outputs ----
    attn = np.empty((Q, H), np.float32)
    key_out = np.empty((1, KVH, C, HD), np.float32)
    value_out = np.empty((1, KVH, C, HD), np.float32)
    for c in range(NCORES):
        r = res.results[c]
        attn[:, c * HPC * HD:(c + 1) * HPC * HD] = (
            r["attn_t"].reshape(HPC * HD, Q).T)
        key_out[0, c, :C - Q] = past_key[0, c, Q:]
        key_out[0, c, C - Q:] = r["k_new"].T
        value_out[0, c, :C - Q] = past_value[0, c, Q:]
        value_out[0, c, C - Q:] = r["v_new"].T
    return attn[None], key_out, value_out


# revision 42
# speedup vs baseline: 1.0865x; 1.0865x over previous
"""Distributed GQA attention kernel for one TRN2 chip (8 NeuronCores).

Problem: B=1, Q=1024 new tokens, C=3072 cached, H=4096, 32 Q heads,
8 KV heads, head_dim=128.  Tensor-parallel over heads: core c owns
Q heads 4c..4c+3 and KV head c.  q/k/v projections column-sharded,
o_proj column-sharded with an AllGather of per-head attention outputs
(equivalent wire cost to row-shard + all-reduce, but transport only —
no CCE arithmetic).  Compute in bf16 (PE runs bf16 at 4x the fp32
rate); accumulation is fp32 in PSUM.

Layout strategy (all host-side prep is free):
  - hidden^T [H+1(bias row), T] so projections contract over partitions
  - past_key^T [hd, kp];  past_value pre-tiled [128, 24, 128]
  - scores computed transposed: s^T[kp, t] = K^T(lhsT) . Q^T(rhs)
  - softmax without max subtraction (scores ~ N(0, ~3.3), exp never
    overflows fp32) and with mask folded in as exp(mask) (host
    precomputed), multiplied post-exp on DVE
  - denominators via ones-matmul on the PE, broadcast back over
    partitions via a 1-partition matmul
"""

import sys

if "/opt/trn_rl_repo" not in sys.path:
    sys.path.insert(0, "/opt/trn_rl_repo")

import numpy as np
import ml_dtypes

BF16 = ml_dtypes.bfloat16

B, Q, C = 1, 1024, 3072
H, NH, KVH = 4096, 32, 8
HD = 128
S = C + Q                  # 4096 key positions
NCORES = 8
HPC = NH // NCORES         # 4 q heads per core
KT = H // 128 + 1          # 33 contraction tiles (incl. bias row tile)
NKP = S // 128             # 32 key-position tiles
CKP = C // 128             # 24 past kv tiles
NTT = Q // 128             # 8 new-token tiles
ISC = 1.0 / np.sqrt(HD)

_CACHE = {}


def _build():
    from concourse import bass, bacc, tile, mybir

    f32 = mybir.dt.float32
    bf16 = mybir.dt.bfloat16
    AF = mybir.ActivationFunctionType

    nc = bacc.Bacc("TRN2", target_bir_lowering=False, debug=False,
                   num_devices=NCORES)

    # ---- I/O ----
    hx_d = nc.dram_tensor("hx", [KT, 128, Q], bf16, kind="ExternalInput")
    wq_d = nc.dram_tensor("wq", [KT, 128, HPC * HD], bf16, kind="ExternalInput")
    wk_d = nc.dram_tensor("wk", [KT, 128, HD], bf16, kind="ExternalInput")
    wv_d = nc.dram_tensor("wv", [KT, 128, HD], bf16, kind="ExternalInput")
    wo_d = nc.dram_tensor("wo", [NH, 128, HPC * HD], bf16, kind="ExternalInput")
    bo_d = nc.dram_tensor("bo_col", [128, HPC], f32, kind="ExternalInput")
    pk_d = nc.dram_tensor("pkT", [128, C], bf16, kind="ExternalInput")
    pv_d = nc.dram_tensor("pv", [128, CKP, 128], bf16, kind="ExternalInput")
    cos_d = nc.dram_tensor("cosT", [128, Q], bf16, kind="ExternalInput")
    sin_d = nc.dram_tensor("ssT", [128, Q], bf16, kind="ExternalInput")
    em_d = nc.dram_tensor("expmT", [NKP, 128, Q], bf16, kind="ExternalInput")
    id_d = nc.dram_tensor("ident", [128, 128], bf16, kind="ExternalInput")

    attn_d = nc.dram_tensor("attn_t", [HPC, 128, Q], f32, kind="ExternalOutput")
    knew_d = nc.dram_tensor("k_new", [128, Q], f32, kind="ExternalOutput")
    vnew_d = nc.dram_tensor("v_new", [128, Q], f32, kind="ExternalOutput")

    with tile.TileContext(nc) as tc:
        with (
            tc.tile_pool(name="const", bufs=1) as cpool,
            tc.tile_pool(name="proj", bufs=1) as proj,
            tc.tile_pool(name="ps", bufs=1, space="PSUM") as ps,
            tc.tile_pool(name="dram", bufs=1, space="DRAM") as dram,
        ):
            # ---- constants (DMAs issued later, behind the hot-path tiles) --
            cos_sb = cpool.tile([128, Q], bf16)
            ss_sb = cpool.tile([128, Q], bf16)
            bo_sb = cpool.tile([128, HPC], f32)
            ident = cpool.tile([128, 128], bf16)
            ones_sb = cpool.tile([128, 1], bf16)
            onesf_sb = cpool.tile([1, 128], bf16)
            zbias = cpool.tile([128, 1], f32)
            nc.vector.memset(ones_sb[:], 1.0)
            nc.vector.memset(onesf_sb[:], 1.0)
            nc.vector.memset(zbias[:], 0.0)

            # ---- persistent per-core tensors ----
            qT_sb = proj.tile([128, HPC, Q], bf16)      # rope'd Q^T per head
            kT_sb = proj.tile([128, S], bf16)           # full K^T (past+new)
            v_sb = proj.tile([128, NKP, 128], bf16)     # full V, kp-tiled

            # ================= phase 1: projections + RoPE =================
            # hidden^T is fully SBUF-resident: streamed in once during the
            # Q pass, reused by the K/V pass with zero extra HBM traffic.
            with (
                tc.tile_pool(name="hxs", bufs=1) as hxp,
                tc.tile_pool(name="wqs", bufs=6) as wqp,
                tc.tile_pool(name="wkvs", bufs=6) as wkvp,
                tc.tile_pool(name="rope", bufs=2) as rp,
            ):
                hx_sb = hxp.tile([128, KT, Q], bf16)

                def rope(dst_bf, ps, also_f32=None):
                    """dst = ps*cos + rotate_half(ps)*ss  (ps is [128, Q] psum)"""
                    a = rp.tile([128, Q], bf16, name="ropa", tag="ropa")
                    b = rp.tile([128, Q], bf16, name="ropb", tag="ropb")
                    nc.vector.tensor_mul(a[:], ps[:], cos_sb[:])
                    nc.vector.tensor_mul(b[0:64, :], ps[64:128, :], ss_sb[0:64, :])
                    nc.vector.tensor_mul(b[64:128, :], ps[0:64, :], ss_sb[64:128, :])
                    nc.vector.tensor_add(dst_bf, a[:], b[:])
                    if also_f32 is not None:
                        nc.vector.tensor_add(also_f32, a[:], b[:])

                # ---- pass A: Q projection, all 4 heads per k-tile ----
                q_ps = [ps.tile([128, Q], f32, name=f"qps{m}", tag=f"S{m}")
                        for m in range(HPC)]
                for k in range(KT):
                    nc.sync.dma_start(hx_sb[:, k, :], hx_d.ap()[k, :, :])
                    wqk = wqp.tile([128, HPC * HD], bf16, name="wqk",
                                   tag="wqk")
                    nc.sync.dma_start(wqk[:], wq_d.ap()[k, :, :])
                    for m in range(HPC):
                        for th in range(2):
                            nc.tensor.matmul(
                                q_ps[m][:, th * 512:(th + 1) * 512],
                                wqk[:, m * 128:(m + 1) * 128],
                                hx_sb[:, k, th * 512:(th + 1) * 512],
                                start=(k == 0), stop=(k == KT - 1))
                # constants + kv cache loads, behind the hot startup path
                nc.sync.dma_start(cos_sb[:], cos_d.ap()[:])
                nc.sync.dma_start(ss_sb[:], sin_d.ap()[:])
                nc.sync.dma_start(ident[:], id_d.ap()[:])
                nc.sync.dma_start(bo_sb[:], bo_d.ap()[:])
                nc.sync.dma_start(kT_sb[:, 0:C], pk_d.ap()[:])
                nc.sync.dma_start(v_sb[:, 0:CKP, :], pv_d.ap()[:])
                for m in range(HPC):
                    rope(qT_sb[:, m, :], q_ps[m])

                # ---- pass B: K and V projections ----
                k_ps = ps.tile([128, Q], f32, name="kps", tag="S0")
                v_ps = ps.tile([128, Q], f32, name="vps", tag="S1")
                for k in range(KT):
                    wkv = wkvp.tile([128, 2 * HD], bf16, name="wkv",
                                    tag="wkv")
                    nc.sync.dma_start(wkv[:, 0:HD], wk_d.ap()[k, :, :])
                    nc.sync.dma_start(wkv[:, HD:2 * HD], wv_d.ap()[k, :, :])
                    for th in range(2):
                        sl = slice(th * 512, (th + 1) * 512)
                        nc.tensor.matmul(k_ps[:, sl], wkv[:, 0:HD],
                                         hx_sb[:, k, sl],
                                         start=(k == 0), stop=(k == KT - 1))
                        nc.tensor.matmul(v_ps[:, sl], wkv[:, HD:2 * HD],
                                         hx_sb[:, k, sl],
                                         start=(k == 0), stop=(k == KT - 1))
                knew_f = rp.tile([128, Q], f32)
                rope(kT_sb[:, C:S], k_ps, also_f32=knew_f[:])
                nc.sync.dma_start(knew_d.ap()[:], knew_f[:])

                vnew_f = rp.tile([128, Q], f32)
                nc.vector.tensor_copy(vnew_f[:], v_ps[:])
                nc.sync.dma_start(vnew_d.ap()[:], vnew_f[:])
                vT_bf = rp.tile([128, Q], bf16)
                nc.vector.tensor_copy(vT_bf[:], v_ps[:])
                # transpose new V into [t, hd] tiles via PE (slots S0/S1 so
                # S2/S3 stay free for head 0's first score matmuls)
                for j in range(NTT):
                    tp = ps.tile([128, 128], bf16, name="vtp",
                                 tag=f"S{j % 2}")
                    nc.tensor.transpose(
                        tp[:], vT_bf[:, j * 128:(j + 1) * 128], ident[:])
                    nc.vector.tensor_copy(v_sb[:, CKP + j, :], tp[:])

            # exp(mask) reuses the SBUF freed by hx; streamed in at the
            # start of phase 2 (phase 2 is PE-bound with DMA headroom),
            # kp-ordered so head 0 stays ahead of the arrivals.
            emres = tc.alloc_tile_pool(name="emres", bufs=1)
            em_sb = emres.tile([128, NKP, Q], bf16)
            for kp in range(NKP):
                nc.sync.dma_start(em_sb[:, kp, :], em_d.ap()[kp, :, :])

            # first AllGather half lands in SBUF mid-phase-2 (right-side
            # stack so it outlives emres, which releases before phase 3)
            agA = tc.alloc_tile_pool(name="agA", bufs=1, side="right")
            agA_sb = agA.tile([128, NH // 2, Q], bf16)

            # ================= phase 2: attention =================
            # AllGather is split in two: heads {0,1} are gathered while
            # heads {2,3} still compute, the second gather overlaps the
            # start of o_proj.
            attg_in = [dram.tile([2, 128, Q], bf16, name=f"aggi{i}")
                       for i in range(2)]
            attg_all = [dram.tile([NH // 2, 128, Q], bf16,
                                  addr_space="Shared", name=f"agga{i}")
                        for i in range(2)]
            att_sb = proj.tile([128, HPC, Q], bf16)
            with (
                tc.tile_pool(name="p2sb", bufs=3) as p2sb,
                tc.tile_pool(name="p2misc", bufs=2) as p2m,
            ):
                for h in range(HPC):
                    av_ps = ps.tile([128, Q], f32, name="av", tag="S0")
                    den_ps = ps.tile([1, Q], f32, name="den", tag="S1")
                    pw_even = None
                    for kp in range(NKP):
                        sc = ps.tile([128, Q], f32, name="sc",
                                     tag=f"S{2 + (kp % 2)}")
                        for th in range(2):
                            nc.tensor.matmul(
                                sc[:, th * 512:(th + 1) * 512],
                                kT_sb[:, kp * 128:(kp + 1) * 128],
                                qT_sb[:, h, th * 512:(th + 1) * 512],
                                start=True, stop=True)
                        pexp = p2sb.tile([128, Q], bf16, name="pexp",
                                         tag="pexp")
                        nc.scalar.activation(pexp[:], sc[:], AF.Exp,
                                             bias=zbias[:], scale=ISC)
                        pw = p2sb.tile([128, Q], bf16, name="pw", tag="pw")
                        nc.vector.tensor_mul(pw[:], pexp[:], em_sb[:, kp, :])
                        first, last = kp == 0, kp == NKP - 1
                        for th in range(2):
                            sl = slice(th * 512, (th + 1) * 512)
                            nc.tensor.matmul(av_ps[:, sl],
                                             v_sb[:, kp, :], pw[:, sl],
                                             start=first, stop=last)
                        if kp % 2 == 0:
                            pw_even = pw
                        else:
                            # denominator: tree-sum pw quads on DVE, then one
                            # ones-matmul per quad (quarters the PE cost)
                            pws = p2sb.tile([128, Q], bf16, name="pws",
                                            tag="pws")
                            nc.vector.tensor_add(pws[:], pw_even[:], pw[:])
                            if kp % 4 == 1:
                                pws_prev = pws
                            else:
                                qws = p2sb.tile([128, Q], bf16, name="qws",
                                                tag="qws")
                                nc.vector.tensor_add(qws[:], pws_prev[:],
                                                     pws[:])
                                for th in range(2):
                                    sl = slice(th * 512, (th + 1) * 512)
                                    nc.tensor.matmul(den_ps[:, sl],
                                                     ones_sb[:], qws[:, sl],
                                                     start=(kp == 3),
                                                     stop=(kp == NKP - 1))
                    # normalize: att = av / den
                    recip = p2m.tile([1, Q], f32, name="recip", tag="recip")
                    nc.vector.reciprocal_approx_fast(recip[:], den_ps[:])
                    recipb = p2m.tile([1, Q], bf16, name="recipb",
                                      tag="recipb")
                    nc.vector.tensor_copy(recipb[:], recip[:])
                    rb_ps = ps.tile([128, Q], f32, name="rb", tag="S1")
                    for th in range(2):
                        nc.tensor.matmul(
                            rb_ps[:, th * 512:(th + 1) * 512],
                            onesf_sb[:],
                            recipb[:, th * 512:(th + 1) * 512],
                            start=True, stop=True)
                    rb_sb = p2m.tile([128, Q], bf16, name="rbsb", tag="rbsb")
                    nc.scalar.copy(rb_sb[:], rb_ps[:])
                    nc.vector.tensor_mul(att_sb[:, h, :], av_ps[:], rb_sb[:])
                    # kick off the half-gather as soon as its heads are done
                    if h == 1 or h == HPC - 1:
                        half = 0 if h == 1 else 1
                        for hi in range(2):
                            nc.sync.dma_start(
                                attg_in[half][hi, :, :],
                                att_sb[:, 2 * half + hi, :])
                        nc.gpsimd.collective_compute(
                            "AllGather",
                            mybir.AluOpType.bypass,
                            replica_groups=[list(range(NCORES))],
                            ins=[attg_in[half].opt()],
                            outs=[attg_all[half].opt()],
                        )
                        if half == 0:
                            # pull gather-half A into SBUF while heads 2-3
                            # still compute
                            for j in range(NH // 2):
                                nc.sync.dma_start(agA_sb[:, j, :],
                                                  attg_all[0][j, :, :])
            emres.release()

            # ================= phase 3: o_proj =================
            # global block j of half i holds head 4*(j//2) + 2*i + j%2;
            # the host reorders Wo blocks to match, so we just iterate.
            with (
                tc.tile_pool(name="p3sb", bufs=1) as p3sb,
                tc.tile_pool(name="wos", bufs=6) as wop,
            ):
                agB_sb = p3sb.tile([128, NH // 2, Q], bf16)
                out_sb = p3sb.tile([128, HPC, Q], f32)
                for j in range(NH // 2):
                    nc.gpsimd.dma_start(agB_sb[:, j, :], attg_all[1][j, :, :])
                op = [ps.tile([128, Q], f32, name=f"op{m}", tag=f"S{m}")
                      for m in range(HPC)]
                for g in range(NH):
                    wog = wop.tile([128, HPC * HD], bf16, name="wog",
                                   tag="wog")
                    nc.sync.dma_start(wog[:], wo_d.ap()[g, :, :])
                    src = agA_sb[:, g, :] if g < NH // 2 \
                        else agB_sb[:, g - NH // 2, :]
                    for m in range(HPC):
                        for th in range(2):
                            sl = slice(th * 512, (th + 1) * 512)
                            nc.tensor.matmul(
                                op[m][:, sl],
                                wog[:, m * 128:(m + 1) * 128],
                                src[:, sl],
                                start=(g == 0), stop=(g == NH - 1))
                for m in range(HPC):
                    nc.scalar.activation(out_sb[:, m, :], op[m][:],
                                         AF.Identity,
                                         bias=bo_sb[:, m:m + 1], scale=1.0)
                    nc.sync.dma_start(attn_d.ap()[m, :, :], out_sb[:, m, :])
            agA.release()

    nc.compile()
    return nc


def _get_nc():
    if "nc" not in _CACHE:
        _CACHE["nc"] = _build()
    return _CACHE["nc"]


def kernel(hidden_states, mask, pos_emb, past_key, past_value,
           Wq, bq, Wk, bk, Wv, bv, Wo, bo):
    from concourse.bass_utils import run_bass_kernel_spmd

    hidden_states = np.asarray(hidden_states, np.float32)
    mask = np.asarray(mask, np.float32)
    pos_emb = np.asarray(pos_emb, np.float32)
    past_key = np.asarray(past_key, np.float32)
    past_value = np.asarray(past_value, np.float32)
    Wq = np.asarray(Wq, np.float32)
    bq = np.asarray(bq, np.float32)
    Wk = np.asarray(Wk, np.float32)
    bk = np.asarray(bk, np.float32)
    Wv = np.asarray(Wv, np.float32)
    bv = np.asarray(bv, np.float32)
    Wo = np.asarray(Wo, np.float32)
    bo = np.asarray(bo, np.float32)

    # ---- shared host prep ----
    hx = np.zeros((KT * 128, Q), np.float32)
    hx[:H] = hidden_states[0].T
    hx[H] = 1.0                                   # bias row
    hx = hx.reshape(KT, 128, Q).astype(BF16)

    cosT = pos_emb[0, 0].T.astype(BF16)           # [128, Q]
    sinT = pos_emb[0, 1].T
    ssT = np.concatenate([-sinT[:64], sinT[64:]]).astype(BF16)

    expmT = np.exp(mask[0, 0]).T.reshape(NKP, 128, Q).astype(BF16)

    # o_proj iterates gathered blocks: g -> head 4*((g%16)//2) + 2*(g//16) + g%2
    wo_order = [4 * ((g % 16) // 2) + 2 * (g // 16) + g % 2 for g in range(NH)]

    def wpad(w, b):
        x = np.zeros((KT * 128, w.shape[1]), np.float32)
        x[:H] = w
        x[H] = b
        return x.reshape(KT, 128, w.shape[1]).astype(BF16)

    in_maps = []
    for c in range(NCORES):
        qs = slice(c * HPC * HD, (c + 1) * HPC * HD)
        ks = slice(c * HD, (c + 1) * HD)
        in_maps.append({
            "hx": hx,
            "wq": wpad(Wq[:, qs], bq[qs]),
            "wk": wpad(Wk[:, ks], bk[ks]),
            "wv": wpad(Wv[:, ks], bv[ks]),
            "wo": Wo[:, qs].reshape(NH, 128, HPC * HD)[wo_order].astype(BF16),
            "bo_col": np.ascontiguousarray(
                bo[qs].reshape(HPC, 128).T.astype(np.float32)),
            "pkT": np.ascontiguousarray(past_key[0, c].T).astype(BF16),
            "pv": np.ascontiguousarray(
                past_value[0, c].reshape(CKP, 128, HD).transpose(1, 0, 2)
            ).astype(BF16),
            "cosT": cosT,
            "ssT": ssT,
            "expmT": expmT,
            "ident": np.eye(128, dtype=BF16),
        })

    nc = _get_nc()
    res = run_bass_kernel_spmd(nc, in_maps, core_ids=list(range(NCORES)))

    # ---- assemble full outputs ----
    attn = np.empty((Q, H), np.float32)
    key_out = np.empty((1, KVH, C, HD), np.float32)
    value_out = np.empty((1, KVH, C, HD), np.float32)
    for c in range(NCORES):
        r = res.results[c]
        attn[:, c * HPC * HD:(c + 1) * HPC * HD] = (
            r["attn_t"].reshape(HPC * HD, Q).T)
        key_out[0, c, :C - Q] = past_key[0, c, Q:]
        key_out[0, c, C - Q:] = r["k_new"].T
        value_out[0, c, :C - Q] = past_value[0, c, Q:]
        value_out[0, c, C - Q:] = r["v_new"].T
    return attn[None], key_out, value_out
